# revision 1
# baseline (speedup 1.0000x reference)
"""Multi-head attention (B=2, S=2048, D=768, H=12) on 8 NeuronCores.

Sharding: data-parallel over batch (2) x tensor-parallel over heads (4 groups
of 3 heads) = 8 cores. Each core computes its 3 heads' Q/K/V projections,
attention, and a partial output projection; the host sums the 4 per-batch
partials and adds the output bias.

Per-core kernel layout (all matmuls in float32r: 1 cycle/row at N>=256):
  xT   [768, 2048]  input transposed (d on partitions, 6 chunks of 128)
  QT,KT[192, 2048]  transposed projections (head-major rows, bias via
                    rank-1 ones matmul)
  V    [2048, 3x65] natural-layout V with a ones column appended per head:
                    the ctx matmul lhsT [sk, 65] then yields softmax
                    denominators in PSUM row 64 for free.
  scoresT [sk 128, sq] per (head, sk-chunk) in PSUM -> Exp on ScalarE
                    (scale=1/sqrt(dk) folded into the activation) -> SBUF
  ctxT accumulates over sk in PSUM [65, 512] per sq-chunk; normalized on
                    eviction via reciprocal_approx_fast + partition_broadcast
  outT [768, 2048] partial output projection, host-summed across head groups
"""

import sys

sys.path.insert(0, "/opt/trn_rl_repo")

import numpy as np

B, S, D = 2, 2048, 768
H, DK = 12, 64
P = 128
HG = 3              # heads per core
E = HG * DK         # 192: per-core projection width
KD = D // P         # 6 contraction chunks
SQC = S // 512      # 4 sq chunks of 512
SKC = S // P        # 16 sk chunks of 128
SCALE = 1.0 / 8.0   # 1/sqrt(DK)

_NC_CACHE = {}


def _build_bass(debug_dumps=False, body_reps=1):
    import concourse.bacc as bacc
    import concourse.tile as tile
    from concourse import mybir

    f32 = mybir.dt.float32
    f32r = mybir.dt.float32r
    Exp = mybir.ActivationFunctionType.Exp

    nc = bacc.Bacc(trn_type="TRN2", debug=False)

    xT = nc.dram_tensor("xT", [D, S], f32, kind="ExternalInput")
    wqT = nc.dram_tensor("wqT", [D, E], f32, kind="ExternalInput")
    wkT = nc.dram_tensor("wkT", [D, E], f32, kind="ExternalInput")
    wvT = nc.dram_tensor("wvT", [D, 256], f32, kind="ExternalInput")
    bq = nc.dram_tensor("bq", [1, E], f32, kind="ExternalInput")
    bk = nc.dram_tensor("bk", [1, E], f32, kind="ExternalInput")
    bv = nc.dram_tensor("bv", [1, 256], f32, kind="ExternalInput")
    woT = nc.dram_tensor("woT", [E, D], f32, kind="ExternalInput")
    ones_d = nc.dram_tensor("ones", [P, 512], f32, kind="ExternalInput")
    outT = nc.dram_tensor("outT", [D, S], f32, kind="ExternalOutput")
    if debug_dumps:
        qt_dump = nc.dram_tensor("qt_dump", [E, S], f32, kind="ExternalOutput")
        kt_dump = nc.dram_tensor("kt_dump", [E, S], f32, kind="ExternalOutput")
        v_dump = nc.dram_tensor("v_dump", [S, HG * 65], f32, kind="ExternalOutput")
        et_dump = nc.dram_tensor("et_dump", [P, 1024], f32, kind="ExternalOutput")
        sc_dump = nc.dram_tensor("sc_dump", [P, 1024], f32, kind="ExternalOutput")
        ctx_dump = nc.dram_tensor("ctx_dump", [E, S], f32, kind="ExternalOutput")
        cps_dump = nc.dram_tensor("cps_dump", [65, 512], f32, kind="ExternalOutput")
        r_dump = nc.dram_tensor("r_dump", [1, 512], f32, kind="ExternalOutput")
        rb_dump = nc.dram_tensor("rb_dump", [64, 512], f32, kind="ExternalOutput")

    xT_d = xT.ap().rearrange("(c p) s -> c p s", p=P)
    wqT_d = wqT.ap().rearrange("(c p) e -> c p e", p=P)
    wkT_d = wkT.ap().rearrange("(c p) e -> c p e", p=P)
    wvT_d = wvT.ap().rearrange("(c p) e -> c p e", p=P)
    outT_d = outT.ap().rearrange("(c p) s -> c p s", p=P)

    with tile.TileContext(nc) as tc:
        for _rep in range(body_reps):
            with tc.tile_pool(name="persist", bufs=1) as persist, \
                 tc.tile_pool(name="work", bufs=4) as work, \
                 tc.tile_pool(name="small", bufs=2) as small, \
                 tc.tile_pool(name="dbg", bufs=2) as dbgp:

                # ---- load inputs (f32r via dtype-punned DMA: PE truncates) ----
                x_sb = []
                for d in range(KD):
                    t = persist.tile([P, S], f32r, tag=f"x{d}")
                    nc.sync.dma_start(out=t[:], in_=xT_d[d].bitcast(f32r))
                    x_sb.append(t)
                wq_sb, wk_sb, wv_sb = [], [], []
                for d in range(KD):
                    t = persist.tile([P, E], f32r, tag=f"wq{d}")
                    nc.sync.dma_start(out=t[:], in_=wqT_d[d].bitcast(f32r))
                    wq_sb.append(t)
                    t = persist.tile([P, E], f32r, tag=f"wk{d}")
                    nc.sync.dma_start(out=t[:], in_=wkT_d[d].bitcast(f32r))
                    wk_sb.append(t)
                    t = persist.tile([P, 256], f32r, tag=f"wv{d}")
                    nc.sync.dma_start(out=t[:], in_=wvT_d[d].bitcast(f32r))
                    wv_sb.append(t)
                bq_sb = persist.tile([1, E], f32r, tag="bq")
                nc.sync.dma_start(out=bq_sb[:], in_=bq.ap().bitcast(f32r))
                bk_sb = persist.tile([1, E], f32r, tag="bk")
                nc.sync.dma_start(out=bk_sb[:], in_=bk.ap().bitcast(f32r))
                bv_sb = persist.tile([1, 256], f32r, tag="bv")
                nc.sync.dma_start(out=bv_sb[:], in_=bv.ap().bitcast(f32r))
                wo_a = persist.tile([P, D], f32r, tag="wo_a")
                nc.sync.dma_start(out=wo_a[:], in_=woT.ap()[0:P, :].bitcast(f32r))
                wo_b = persist.tile([64, D], f32r, tag="wo_b")
                nc.sync.dma_start(out=wo_b[:], in_=woT.ap()[P:E, :].bitcast(f32r))

                ones = persist.tile([P, 512], f32r, tag="ones")
                nc.sync.dma_start(out=ones[:], in_=ones_d.ap().bitcast(f32r))

                # ---- persistent activations ----
                qt_a = persist.tile([P, S], f32r, tag="qt_a")   # heads 0,1
                qt_b = persist.tile([64, S], f32r, tag="qt_b")  # head 2
                kt_a = persist.tile([P, S], f32r, tag="kt_a")
                kt_b = persist.tile([64, S], f32r, tag="kt_b")
                v_sb = [persist.tile([P, HG, 65], f32r, tag=f"v{i}", name=f"v{i}") for i in range(SKC)]
                ctx_a = persist.tile([P, S], f32r, tag="ctx_a")
                ctx_b = persist.tile([64, S], f32r, tag="ctx_b")

                # ================= QKV projections =================
                with tc.tile_pool(name="proj_ps", bufs=8, space="PSUM") as proj_ps:
                    for (w_chunks, b_tile, dst_a, dst_b) in (
                        (wq_sb, bq_sb, qt_a, qt_b),
                        (wk_sb, bk_sb, kt_a, kt_b),
                    ):
                        ps = []
                        for m in range(2):  # e-tiles: [0:128], [128:192]
                            mw = P if m == 0 else 64
                            for c in range(SQC):
                                ps.append(proj_ps.tile([mw, 512], f32, tag="proj", name=f"proj_ps_{m}_{c}"))
                        for d in range(KD):
                            k = 0
                            for m in range(2):
                                mw = P if m == 0 else 64
                                for c in range(SQC):
                                    nc.tensor.matmul(
                                        ps[k][:],
                                        w_chunks[d][:, m * P : m * P + mw],
                                        x_sb[d][:, c * 512 : (c + 1) * 512],
                                        start=(d == 0), stop=False,
                                    )
                                    k += 1
                        k = 0
                        for m in range(2):
                            mw = P if m == 0 else 64
                            for c in range(SQC):
                                nc.tensor.matmul(
                                    ps[k][:],
                                    b_tile[0:1, m * P : m * P + mw],
                                    ones[0:1, 0:512],
                                    start=False, stop=True,
                                )
                                k += 1
                        k = 0
                        for m in range(2):
                            mw = P if m == 0 else 64
                            dst = dst_a if m == 0 else dst_b
                            for c in range(SQC):
                                nc.vector.tensor_copy(
                                    dst[0:mw, c * 512 : (c + 1) * 512], ps[k][:]
                                )
                                k += 1

                    if debug_dumps:
                        nc.sync.dma_start(out=qt_dump.ap()[0:P, :].bitcast(f32r), in_=qt_a[:])
                        nc.sync.dma_start(out=qt_dump.ap()[P:E, :].bitcast(f32r), in_=qt_b[:])
                        nc.sync.dma_start(out=kt_dump.ap()[0:P, :].bitcast(f32r), in_=kt_a[:])
                        nc.sync.dma_start(out=kt_dump.ap()[P:E, :].bitcast(f32r), in_=kt_b[:])


                if debug_dumps:
                    v_dump_d = v_dump.ap().rearrange("(i p) m -> i p m", p=P)
                    for i in range(SKC):
                        nc.sync.dma_start(
                            out=v_dump_d[i].bitcast(f32r),
                            in_=v_sb[i][:].rearrange("p h m -> p (h m)"),
                        )

                # ================= attention =================
                with tc.tile_pool(name="sc_ps", bufs=2, space="PSUM") as sc_ps, \
                     tc.tile_pool(name="ctx_ps", bufs=4, space="PSUM") as ctx_ps:
                    for h in range(HG):
                        if h < 2:
                            kt_h = kt_a[h * 64 : (h + 1) * 64, :]
                            qt_h = qt_a[h * 64 : (h + 1) * 64, :]
                            ctx_h = ctx_a[h * 64 : (h + 1) * 64, :]
                        else:
                            kt_h = kt_b[0:64, :]
                            qt_h = qt_b[0:64, :]
                            ctx_h = ctx_b[0:64, :]

                        cps = [ctx_ps.tile([65, 512], f32, tag="ctx", name=f"cps_{h}_{c}") for c in range(SQC)]
                        for i in range(SKC):
                            sps, ets = [], []
                            for half in range(2):
                                sp = sc_ps.tile([P, 1024], f32, tag="sc", name=f"sp_{h}_{i}_{half}")
                                for j in range(2):
                                    nc.tensor.matmul(
                                        sp[:, j * 512 : (j + 1) * 512],
                                        kt_h[:, i * P : (i + 1) * P],
                                        qt_h[:, half * 1024 + j * 512 : half * 1024 + (j + 1) * 512],
                                        start=True, stop=True,
                                    )
                                sps.append(sp)
                            if h == 0:
                                # V projection for sk-tile i, interleaved into
                                # the ACT-paced attention pipeline (PE slack)
                                vps = sc_ps.tile([P, 256], f32, tag="sc", name=f"vps_{i}")
                                for d in range(KD):
                                    nc.tensor.matmul(
                                        vps[:],
                                        x_sb[d][:, i * P : (i + 1) * P],
                                        wv_sb[d][:],
                                        start=(d == 0), stop=False,
                                    )
                                nc.tensor.matmul(
                                    vps[:], ones[0:1, 0:P], bv_sb[0:1, :],
                                    start=False, stop=True,
                                )
                                nc.vector.tensor_copy(
                                    v_sb[i][:, :, 64:65], ones[:, 0:3][:, :, None]
                                )
                                nc.vector.tensor_copy(
                                    v_sb[i][:, :, 0:64],
                                    vps[:, 0:E].rearrange("p (h d) -> p h d", h=HG),
                                )
                            for half in range(2):
                                et = work.tile([P, 1024], f32r, tag="exp", name=f"et_{h}_{i}_{half}")
                                if debug_dumps and h == 0 and i == 0 and half == 0:
                                    scd = dbgp.tile([P, 1024], f32, tag="scd", name="scd")
                                    nc.vector.tensor_copy(scd[:], sps[half][:])
                                    nc.sync.dma_start(out=sc_dump.ap(), in_=scd[:])
                                nc.scalar.activation(et[:], sps[half][:], Exp, scale=SCALE)
                                if debug_dumps and h == 0 and i == 0 and half == 0:
                                    nc.sync.dma_start(out=et_dump.ap().bitcast(f32r), in_=et[:])
                                ets.append(et)
                            for half in range(2):
                                for j in range(2):
                                    c = half * 2 + j
                                    nc.tensor.matmul(
                                        cps[c][:],
                                        v_sb[i][:, h, :],
                                        ets[half][:, j * 512 : (j + 1) * 512],
                                        start=(i == 0), stop=(i == SKC - 1),
                                    )
                        if debug_dumps and h == 0:
                            cpd = dbgp.tile([65, 512], f32, tag="cpd", name="cpd")
                            nc.vector.tensor_copy(cpd[:], cps[0][:])
                            nc.sync.dma_start(out=cps_dump.ap(), in_=cpd[:])
                        for c in range(SQC):
                            den = small.tile([1, 512], f32, tag="den")
                            nc.vector.tensor_copy(den[:], cps[c][64:65, :])
                            r = small.tile([1, 512], f32, tag="r")
                            nc.vector.reciprocal_approx_fast(r[:], den[:])
                            rb = small.tile([64, 512], f32, tag="rb")
                            nc.gpsimd.partition_broadcast(rb[:], r[:])
                            nc.vector.tensor_mul(
                                ctx_h[:, c * 512 : (c + 1) * 512],
                                cps[c][0:64, :],
                                rb[:],
                            )
                            if debug_dumps and h == 0 and c == 0:
                                nc.sync.dma_start(out=r_dump.ap(), in_=r[:])
                                nc.sync.dma_start(out=rb_dump.ap(), in_=rb[:])
                            if h == HG - 1:
                                # output projection for this sq chunk (all heads done)
                                for e in range(KD):
                                    op = ctx_ps.tile([P, 512], f32, tag="ctx",
                                                     name=f"op_{e}_{c}")
                                    nc.tensor.matmul(
                                        op[:],
                                        wo_a[:, e * P : (e + 1) * P],
                                        ctx_a[:, c * 512 : (c + 1) * 512],
                                        start=True, stop=False,
                                    )
                                    nc.tensor.matmul(
                                        op[:],
                                        wo_b[:, e * P : (e + 1) * P],
                                        ctx_b[:, c * 512 : (c + 1) * 512],
                                        start=False, stop=True,
                                    )
                                    o = work.tile([P, 512], f32, tag="o", bufs=6)
                                    if e % 2 == 0:
                                        nc.vector.tensor_copy(o[:], op[:])
                                    else:
                                        nc.scalar.activation(
                                            o[:], op[:],
                                            mybir.ActivationFunctionType.Copy,
                                        )
                                    nc.sync.dma_start(
                                        out=outT_d[e][:, c * 512 : (c + 1) * 512], in_=o[:]
                                    )

                if debug_dumps:
                    nc.sync.dma_start(out=ctx_dump.ap()[0:P, :].bitcast(f32r), in_=ctx_a[:])
                    nc.sync.dma_start(out=ctx_dump.ap()[P:E, :].bitcast(f32r), in_=ctx_b[:])


    nc.finalize()
    return nc


def _get_nc(debug_dumps=False, body_reps=1):
    key = ("dbg" if debug_dumps else "nc", body_reps)
    if key not in _NC_CACHE:
        _NC_CACHE[key] = _build_bass(debug_dumps, body_reps)
    return _NC_CACHE[key]


def _core_inputs(c, x, w_q, b_q, w_k, b_k, w_v, b_v, w_o):
    b, g = divmod(c, 4)
    gs = slice(g * E, (g + 1) * E)
    wv_pad = np.zeros((D, 256), np.float32)
    wv_pad[:, :E] = np.ascontiguousarray(w_v[gs, :].T)
    bv_pad = np.zeros((1, 256), np.float32)
    bv_pad[0, :E] = b_v[gs]
    return {
        "xT": np.ascontiguousarray(x[b].T),
        "wqT": np.ascontiguousarray(w_q[gs, :].T),
        "wkT": np.ascontiguousarray(w_k[gs, :].T),
        "wvT": wv_pad,
        "bq": b_q[gs].reshape(1, E).astype(np.float32),
        "bk": b_k[gs].reshape(1, E).astype(np.float32),
        "bv": bv_pad,
        "woT": np.ascontiguousarray(w_o[:, gs].T),
        "ones": np.ones((P, 512), np.float32),
    }


def kernel(x, w_q, b_q, w_k, b_k, w_v, b_v, w_o, b_o, _trace=False, _debug=False):
    from concourse.bass_utils import run_bass_kernel_spmd

    x = np.asarray(x, np.float32)
    args = [np.asarray(a, np.float32) for a in
            (w_q, b_q, w_k, b_k, w_v, b_v, w_o)]
    b_o = np.asarray(b_o, np.float32)

    nc = _get_nc(_debug)
    in_maps = [_core_inputs(c, x, *args) for c in range(8)]
    res = run_bass_kernel_spmd(nc, in_maps, core_ids=list(range(8)), trace=_trace)

    out = np.zeros((B, S, D), np.float32)
    for c in range(8):
        out[c // 4] += res.results[c]["outT"].T
    out += b_o
    if _trace:
        kernel._last_results = res
    return out



# revision 2
# speedup vs baseline: 1.3343x; 1.3343x over previous
"""Multi-head attention (B=2, S=2048, D=768, H=12) on 8 NeuronCores.

Sharding: data-parallel over batch (2) x tensor-parallel over heads (4 groups
of 3 heads) = 8 cores. Each core computes its 3 heads' Q/K/V projections,
attention, and a partial output projection; the host sums the 4 per-batch
partials and adds the output bias.

All SBUF operands are fp16 (PE fast mode + FWL; PSUM accumulation stays
fp32), halving DMA traffic vs fp32. Per-core kernel layout:
  xT   [768, 2048]  input transposed (d on partitions, 6 chunks of 128)
  QT,KT[192, 2048]  transposed projections (head-major rows, bias via
                    rank-1 ones matmul)
  V    [2048, 3x65] natural-layout V with a ones column appended per head:
                    the ctx matmul lhsT [sk, 65] then yields softmax
                    denominators in PSUM row 64 for free.
  scoresT [sk 128, sq] per (head, sk-chunk) in PSUM -> Exp on ScalarE
                    (scale=1/sqrt(dk) folded into the activation) -> SBUF
  ctxT accumulates over sk in PSUM [65, 512] per sq-chunk; normalized on
                    eviction via reciprocal_approx_fast + partition_broadcast
  outT [768, 2048] fp32 partial output projection, host-summed across
                    head groups
"""

import sys

sys.path.insert(0, "/opt/trn_rl_repo")

import numpy as np

B, S, D = 2, 2048, 768
H, DK = 12, 64
P = 128
HG = 3              # heads per core
E = HG * DK         # 192: per-core projection width
KD = D // P         # 6 contraction chunks
SQC = S // 512      # 4 sq chunks of 512
SKC = S // P        # 16 sk chunks of 128
SCALE = 1.0 / 8.0   # 1/sqrt(DK)

_NC_CACHE = {}


def _build_bass(body_reps=1):
    import concourse.bacc as bacc
    import concourse.tile as tile
    from concourse import mybir

    f16 = mybir.dt.float16
    f32 = mybir.dt.float32
    Exp = mybir.ActivationFunctionType.Exp

    nc = bacc.Bacc(trn_type="TRN2", debug=False)

    xT = nc.dram_tensor("xT", [D, S], f16, kind="ExternalInput")
    wqT = nc.dram_tensor("wqT", [D, E], f16, kind="ExternalInput")
    wkT = nc.dram_tensor("wkT", [D, E], f16, kind="ExternalInput")
    wvT = nc.dram_tensor("wvT", [D, E], f16, kind="ExternalInput")
    bq = nc.dram_tensor("bq", [1, E], f16, kind="ExternalInput")
    bk = nc.dram_tensor("bk", [1, E], f16, kind="ExternalInput")
    bv = nc.dram_tensor("bv", [1, E], f16, kind="ExternalInput")
    woT = nc.dram_tensor("woT", [E, D], f16, kind="ExternalInput")
    ones_d = nc.dram_tensor("ones", [P, 512], f16, kind="ExternalInput")
    outT = nc.dram_tensor("outT", [D, S], f32, kind="ExternalOutput")

    xT_d = xT.ap().rearrange("(c p) s -> c p s", p=P)
    wqT_d = wqT.ap().rearrange("(c p) e -> c p e", p=P)
    wkT_d = wkT.ap().rearrange("(c p) e -> c p e", p=P)
    wvT_d = wvT.ap().rearrange("(c p) e -> c p e", p=P)
    outT_d = outT.ap().rearrange("(c p) s -> c p s", p=P)

    with tile.TileContext(nc) as tc:
        for _rep in range(body_reps):
            with tc.tile_pool(name="persist", bufs=1) as persist, \
                 tc.tile_pool(name="work", bufs=4) as work, \
                 tc.tile_pool(name="small", bufs=2) as small:

                # ---- load inputs (weights first, then x) ----
                wq_sb, wk_sb, wv_sb = [], [], []
                for d in range(KD):
                    t = persist.tile([P, E], f16, tag=f"wq{d}")
                    nc.sync.dma_start(out=t[:], in_=wqT_d[d])
                    wq_sb.append(t)
                    t = persist.tile([P, E], f16, tag=f"wk{d}")
                    nc.sync.dma_start(out=t[:], in_=wkT_d[d])
                    wk_sb.append(t)
                    t = persist.tile([P, E], f16, tag=f"wv{d}")
                    nc.sync.dma_start(out=t[:], in_=wvT_d[d])
                    wv_sb.append(t)
                bq_sb = persist.tile([1, E], f16, tag="bq")
                nc.sync.dma_start(out=bq_sb[:], in_=bq.ap())
                bk_sb = persist.tile([1, E], f16, tag="bk")
                nc.sync.dma_start(out=bk_sb[:], in_=bk.ap())
                bv_sb = persist.tile([1, E], f16, tag="bv")
                nc.sync.dma_start(out=bv_sb[:], in_=bv.ap())
                wo_a = persist.tile([P, D], f16, tag="wo_a")
                nc.sync.dma_start(out=wo_a[:], in_=woT.ap()[0:P, :])
                wo_b = persist.tile([64, D], f16, tag="wo_b")
                nc.sync.dma_start(out=wo_b[:], in_=woT.ap()[P:E, :])
                ones = persist.tile([P, 512], f16, tag="ones")
                nc.sync.dma_start(out=ones[:], in_=ones_d.ap())

                x_sb = []
                for d in range(KD):
                    t = persist.tile([P, S], f16, tag=f"x{d}")
                    nc.sync.dma_start(out=t[:], in_=xT_d[d])
                    x_sb.append(t)

                # preload the Exp table while the x DMAs run
                warm = small.tile([1, 16], f16, tag="warm")
                nc.scalar.activation(warm[:], ones[0:1, 0:16], Exp, scale=1.0)

                # ---- persistent activations ----
                qt_a = persist.tile([P, S], f16, tag="qt_a")   # heads 0,1
                qt_b = persist.tile([64, S], f16, tag="qt_b")  # head 2
                kt_a = persist.tile([P, S], f16, tag="kt_a")
                kt_b = persist.tile([64, S], f16, tag="kt_b")
                v_sb = [persist.tile([P, HG, 65], f16, tag=f"v{i}", name=f"v{i}")
                        for i in range(SKC)]
                ctx_a = persist.tile([P, S], f16, tag="ctx_a")
                ctx_b = persist.tile([64, S], f16, tag="ctx_b")

                # ================= Q/K projections =================
                with tc.tile_pool(name="proj_ps", bufs=8, space="PSUM") as proj_ps:
                    for (w_chunks, b_tile, dst_a, dst_b) in (
                        (wq_sb, bq_sb, qt_a, qt_b),
                        (wk_sb, bk_sb, kt_a, kt_b),
                    ):
                        ps = []
                        for m in range(2):  # e-tiles: [0:128], [128:192]
                            mw = P if m == 0 else 64
                            for c in range(SQC):
                                ps.append(proj_ps.tile([mw, 512], f32, tag="proj",
                                                       name=f"proj_ps_{m}_{c}"))
                        for d in range(KD):
                            k = 0
                            for m in range(2):
                                mw = P if m == 0 else 64
                                for c in range(SQC):
                                    nc.tensor.matmul(
                                        ps[k][:],
                                        w_chunks[d][:, m * P : m * P + mw],
                                        x_sb[d][:, c * 512 : (c + 1) * 512],
                                        start=(d == 0), stop=False,
                                    )
                                    k += 1
                        k = 0
                        for m in range(2):
                            mw = P if m == 0 else 64
                            for c in range(SQC):
                                nc.tensor.matmul(
                                    ps[k][:],
                                    b_tile[0:1, m * P : m * P + mw],
                                    ones[0:1, 0:512],
                                    start=False, stop=True,
                                )
                                k += 1
                        k = 0
                        for m in range(2):
                            mw = P if m == 0 else 64
                            dst = dst_a if m == 0 else dst_b
                            for c in range(SQC):
                                nc.vector.tensor_copy(
                                    dst[0:mw, c * 512 : (c + 1) * 512], ps[k][:]
                                )
                                k += 1

                # ================= attention =================
                with tc.tile_pool(name="sc_ps", bufs=2, space="PSUM") as sc_ps, \
                     tc.tile_pool(name="ctx_ps", bufs=4, space="PSUM") as ctx_ps:
                    for h in range(HG):
                        if h < 2:
                            kt_h = kt_a[h * 64 : (h + 1) * 64, :]
                            qt_h = qt_a[h * 64 : (h + 1) * 64, :]
                            ctx_h = ctx_a[h * 64 : (h + 1) * 64, :]
                        else:
                            kt_h = kt_b[0:64, :]
                            qt_h = qt_b[0:64, :]
                            ctx_h = ctx_b[0:64, :]

                        cps = [ctx_ps.tile([65, 512], f32, tag="ctx",
                                           name=f"cps_{h}_{c}") for c in range(SQC)]
                        for i in range(SKC):
                            sps, ets = [], []
                            for half in range(2):
                                sp = sc_ps.tile([P, 1024], f32, tag="sc",
                                                name=f"sp_{h}_{i}_{half}")
                                for j in range(2):
                                    nc.tensor.matmul(
                                        sp[:, j * 512 : (j + 1) * 512],
                                        kt_h[:, i * P : (i + 1) * P],
                                        qt_h[:, half * 1024 + j * 512 : half * 1024 + (j + 1) * 512],
                                        start=True, stop=True,
                                    )
                                sps.append(sp)
                            if h == 0:
                                # V projection for sk-tile i, interleaved into
                                # the ACT-paced attention pipeline (PE slack)
                                vps = sc_ps.tile([P, E], f32, tag="sc", name=f"vps_{i}")
                                for d in range(KD):
                                    nc.tensor.matmul(
                                        vps[:],
                                        x_sb[d][:, i * P : (i + 1) * P],
                                        wv_sb[d][:],
                                        start=(d == 0), stop=False,
                                    )
                                nc.tensor.matmul(
                                    vps[:], ones[0:1, 0:P], bv_sb[0:1, :],
                                    start=False, stop=True,
                                )
                                nc.vector.tensor_copy(
                                    v_sb[i][:, :, 64:65], ones[:, 0:3][:, :, None]
                                )
                                nc.vector.tensor_copy(
                                    v_sb[i][:, :, 0:64],
                                    vps[:, 0:E].rearrange("p (h d) -> p h d", h=HG),
                                )
                            for half in range(2):
                                et = work.tile([P, 1024], f16, tag="exp",
                                               name=f"et_{h}_{i}_{half}")
                                nc.scalar.activation(et[:], sps[half][:], Exp,
                                                     scale=SCALE)
                                ets.append(et)
                            for half in range(2):
                                for j in range(2):
                                    c = half * 2 + j
                                    nc.tensor.matmul(
                                        cps[c][:],
                                        v_sb[i][:, h, :],
                                        ets[half][:, j * 512 : (j + 1) * 512],
                                        start=(i == 0), stop=(i == SKC - 1),
                                    )
                        for c in range(SQC):
                            den = small.tile([1, 512], f32, tag="den")
                            nc.vector.tensor_copy(den[:], cps[c][64:65, :])
                            r = small.tile([1, 512], f32, tag="r")
                            nc.vector.reciprocal_approx_fast(r[:], den[:])
                            rb = small.tile([64, 512], f32, tag="rb")
                            nc.gpsimd.partition_broadcast(rb[:], r[:])
                            nc.vector.tensor_mul(
                                ctx_h[:, c * 512 : (c + 1) * 512],
                                cps[c][0:64, :],
                                rb[:],
                            )
                            if h == HG - 1:
                                # output projection for this sq chunk (all heads done)
                                for e in range(KD):
                                    op = ctx_ps.tile([P, 512], f32, tag="ctx",
                                                     name=f"op_{e}_{c}")
                                    nc.tensor.matmul(
                                        op[:],
                                        wo_a[:, e * P : (e + 1) * P],
                                        ctx_a[:, c * 512 : (c + 1) * 512],
                                        start=True, stop=False,
                                    )
                                    nc.tensor.matmul(
                                        op[:],
                                        wo_b[:, e * P : (e + 1) * P],
                                        ctx_b[:, c * 512 : (c + 1) * 512],
                                        start=False, stop=True,
                                    )
                                    o = work.tile([P, 512], f32, tag="o", bufs=6)
                                    if e % 2 == 0:
                                        nc.vector.tensor_copy(o[:], op[:])
                                    else:
                                        nc.scalar.activation(
                                            o[:], op[:],
                                            mybir.ActivationFunctionType.Copy,
                                        )
                                    nc.sync.dma_start(
                                        out=outT_d[e][:, c * 512 : (c + 1) * 512],
                                        in_=o[:],
                                    )

    nc.finalize()
    return nc


def _get_nc(body_reps=1):
    key = ("nc", body_reps)
    if key not in _NC_CACHE:
        _NC_CACHE[key] = _build_bass(body_reps)
    return _NC_CACHE[key]


def _core_inputs(c, x, w_q, b_q, w_k, b_k, w_v, b_v, w_o):
    b, g = divmod(c, 4)
    gs = slice(g * E, (g + 1) * E)
    return {
        "xT": np.ascontiguousarray(x[b].T).astype(np.float16),
        "wqT": np.ascontiguousarray(w_q[gs, :].T).astype(np.float16),
        "wkT": np.ascontiguousarray(w_k[gs, :].T).astype(np.float16),
        "wvT": np.ascontiguousarray(w_v[gs, :].T).astype(np.float16),
        "bq": b_q[gs].reshape(1, E).astype(np.float16),
        "bk": b_k[gs].reshape(1, E).astype(np.float16),
        "bv": b_v[gs].reshape(1, E).astype(np.float16),
        "woT": np.ascontiguousarray(w_o[:, gs].T).astype(np.float16),
        "ones": np.ones((P, 512), np.float16),
    }


def kernel(x, w_q, b_q, w_k, b_k, w_v, b_v, w_o, b_o, _trace=False, _debug=False):
    from concourse.bass_utils import run_bass_kernel_spmd

    x = np.asarray(x, np.float32)
    args = [np.asarray(a, np.float32) for a in
            (w_q, b_q, w_k, b_k, w_v, b_v, w_o)]
    b_o = np.asarray(b_o, np.float32)

    nc = _get_nc()
    in_maps = [_core_inputs(c, x, *args) for c in range(8)]
    res = run_bass_kernel_spmd(nc, in_maps, core_ids=list(range(8)), trace=_trace)

    out = np.zeros((B, S, D), np.float32)
    for c in range(8):
        out[c // 4] += res.results[c]["outT"].T
    out += b_o
    if _trace:
        kernel._last_results = res
    return out


# revision 4
# speedup vs baseline: 1.3996x; 1.0489x over previous
"""Multi-head attention (B=2, S=2048, D=768, H=12) on 8 NeuronCores.

Sharding: data-parallel over batch (2) x tensor-parallel over heads (4 groups
of 3 heads) = 8 cores. Each core computes its 3 heads' Q/K/V projections,
attention, and a partial output projection; the host sums the 4 per-batch
partials and adds the output bias.

All SBUF operands are fp16 (PE fast mode + FWL; PSUM accumulation stays
fp32), halving DMA traffic vs fp32. The schedule keeps ScalarE (the Exp
pipeline, ~110us floor) busy end-to-end and hides everything else in the
PE's slack behind it:
  - batched DMAs (one descriptor per logical tensor; x in two pieces)
  - only the m=0 (heads 0,1) Q/K tiles are projected up front -- attention
    for head 0 starts ~17us in; the m=1 (head 2) tiles and the V projection
    are interleaved into the head-0/1 attention loops, drawing PSUM from
    the ctx pool's spare slots
  - attention is split by sq column pairs (2x 1024 cols): scoresT
    [sk 128, 1024] per (cpair, head, sk-chunk) in PSUM -> one Exp on
    ScalarE (scale folded in) -> ctx accumulation [65, 512] with a ones
    column in V giving softmax denominators for free; the cpair-0 output
    projection overlaps cpair-1 attention, shrinking the tail
  outT [768, 2048] fp32 partial output projection, host-summed across
    head groups
"""

import sys

sys.path.insert(0, "/opt/trn_rl_repo")

import numpy as np

B, S, D = 2, 2048, 768
H, DK = 12, 64
P = 128
HG = 3              # heads per core
E = HG * DK         # 192: per-core projection width
KD = D // P         # 6 contraction chunks
SQC = S // 512      # 4 sq chunks of 512
SKC = S // P        # 16 sk chunks of 128
SCALE = 1.0 / 8.0   # 1/sqrt(DK)

_NC_CACHE = {}


def _build_bass(body_reps=1):
    import concourse.bacc as bacc
    import concourse.tile as tile
    from concourse import mybir

    f16 = mybir.dt.float16
    f32 = mybir.dt.float32
    Exp = mybir.ActivationFunctionType.Exp

    nc = bacc.Bacc(trn_type="TRN2", debug=False)

    xT = nc.dram_tensor("xT", [D, S], f16, kind="ExternalInput")
    wqkT = nc.dram_tensor("wqkT", [D, 2 * E], f16, kind="ExternalInput")
    wvT = nc.dram_tensor("wvT", [D, E], f16, kind="ExternalInput")
    bqkv = nc.dram_tensor("bqkv", [1, 3 * E], f16, kind="ExternalInput")
    woT = nc.dram_tensor("woT", [E, D], f16, kind="ExternalInput")
    ones_d = nc.dram_tensor("ones", [P, 512], f16, kind="ExternalInput")
    outT = nc.dram_tensor("outT", [D, S], f32, kind="ExternalOutput")

    xT_d = xT.ap().rearrange("(c p) s -> p c s", p=P)
    wqkT_d = wqkT.ap().rearrange("(c p) e -> p c e", p=P)
    wvT_d = wvT.ap().rearrange("(c p) e -> p c e", p=P)
    outT_d = outT.ap().rearrange("(c p) s -> c p s", p=P)

    with tile.TileContext(nc) as tc:
        for _rep in range(body_reps):
            with tc.tile_pool(name="persist", bufs=1) as persist, \
                 tc.tile_pool(name="work", bufs=4) as work, \
                 tc.tile_pool(name="small", bufs=2) as small:

                # ---- batched input DMAs ----
                wqk = persist.tile([P, KD, 2 * E], f16, tag="wqk")
                nc.sync.dma_start(out=wqk[:], in_=wqkT_d)
                bqkv_sb = persist.tile([1, 3 * E], f16, tag="bqkv")
                nc.sync.dma_start(out=bqkv_sb[:], in_=bqkv.ap())
                ones = persist.tile([P, 512], f16, tag="ones")
                nc.sync.dma_start(out=ones[:], in_=ones_d.ap())
                x_all = persist.tile([P, KD, S], f16, tag="x")
                nc.sync.dma_start(out=x_all[:, 0:3, :], in_=xT_d[:, 0:3, :])
                nc.sync.dma_start(out=x_all[:, 3:KD, :], in_=xT_d[:, 3:KD, :])
                wv = persist.tile([P, KD, E], f16, tag="wv")
                nc.sync.dma_start(out=wv[:], in_=wvT_d)
                wo_a = persist.tile([P, D], f16, tag="wo_a")
                nc.sync.dma_start(out=wo_a[:], in_=woT.ap()[0:P, :])
                wo_b = persist.tile([64, D], f16, tag="wo_b")
                nc.sync.dma_start(out=wo_b[:], in_=woT.ap()[P:E, :])

                # preload the Exp table while the x DMAs run
                warm = small.tile([1, 16], f16, tag="warm")
                nc.scalar.activation(warm[:], ones[0:1, 0:16], Exp, scale=1.0)

                # ---- persistent activations ----
                qt_a = persist.tile([P, S], f16, tag="qt_a")   # heads 0,1
                qt_b = persist.tile([64, S], f16, tag="qt_b")  # head 2
                kt_a = persist.tile([P, S], f16, tag="kt_a")
                kt_b = persist.tile([64, S], f16, tag="kt_b")
                v_sb = [persist.tile([P, HG, 65], f16, tag=f"v{i}", name=f"v{i}")
                        for i in range(SKC)]
                ctx_a = persist.tile([P, S], f16, tag="ctx_a")
                ctx_b = persist.tile([64, S], f16, tag="ctx_b")

                def w_slice(d, which, m, mw):
                    off = 0 if which == 0 else E
                    return wqk[:, d, off + m * P : off + m * P + mw]

                def b_slice(which, m, mw):
                    off = which * E
                    return bqkv_sb[0:1, off + m * P : off + m * P + mw]

                # ====== Q/K m=0 projections (heads 0,1) ======
                # d=0..2 passes first (first x DMA piece), then d=3..5.
                with tc.tile_pool(name="proj_ps", bufs=8, space="PSUM") as proj_ps:
                    ps = []
                    for which in range(2):
                        for c in range(SQC):
                            ps.append(proj_ps.tile(
                                [P, 512], f32, tag="proj",
                                name=f"proj_{which}_{c}"))
                    for dlo, dhi in ((0, 3), (3, KD)):
                        k = 0
                        for which in range(2):
                            for c in range(SQC):
                                for d in range(dlo, dhi):
                                    nc.tensor.matmul(
                                        ps[k][:],
                                        w_slice(d, which, 0, P),
                                        x_all[:, d, c * 512 : (c + 1) * 512],
                                        start=(d == 0), stop=False,
                                    )
                                k += 1
                    k = 0
                    for which, dst in ((0, qt_a), (1, kt_a)):
                        for c in range(SQC):
                            nc.tensor.matmul(
                                ps[k][:],
                                b_slice(which, 0, P),
                                ones[0:1, 0:512],
                                start=False, stop=True,
                            )
                            nc.vector.tensor_copy(
                                dst[:, c * 512 : (c + 1) * 512], ps[k][:]
                            )
                            k += 1

                # ====== attention ======
                # PE slack behind the ACT-bound exp pipeline absorbs the V
                # projection (head-0 loop), the m=1 Q/K projection (head-1
                # loop), and the cpair-0 output projection (head-2 loop).
                def v_proj(i, pool):
                    vps = pool.tile([P, E], f32, tag="ctx", name=f"vps_{i}")
                    for d in range(KD):
                        nc.tensor.matmul(
                            vps[:],
                            x_all[:, d, i * P : (i + 1) * P],
                            wv[:, d, :],
                            start=(d == 0), stop=False,
                        )
                    nc.tensor.matmul(
                        vps[:], ones[0:1, 0:P], b_slice(2, 0, E),
                        start=False, stop=True,
                    )
                    nc.vector.tensor_copy(
                        v_sb[i][:, :, 64:65], ones[:, 0:3][:, :, None]
                    )
                    nc.vector.tensor_copy(
                        v_sb[i][:, :, 0:64],
                        vps[:, 0:E].rearrange("p (h d) -> p h d", h=HG),
                    )

                def m1_proj(t, pool):
                    # t in 0..7: Q m=1 tiles (c=0..3) then K m=1 tiles
                    which, c = divmod(t, SQC)
                    dst = qt_b if which == 0 else kt_b
                    mp = pool.tile([64, 512], f32, tag="ctx", name=f"m1_{t}")
                    for d in range(KD):
                        nc.tensor.matmul(
                            mp[:],
                            w_slice(d, which, 1, 64),
                            x_all[:, d, c * 512 : (c + 1) * 512],
                            start=(d == 0), stop=False,
                        )
                    nc.tensor.matmul(
                        mp[:], b_slice(which, 1, 64), ones[0:1, 0:512],
                        start=False, stop=True,
                    )
                    nc.vector.tensor_copy(dst[:, c * 512 : (c + 1) * 512], mp[:])

                with tc.tile_pool(name="sc_ps", bufs=2, space="PSUM") as sc_ps, \
                     tc.tile_pool(name="ctx_ps", bufs=4, space="PSUM") as ctx_ps:
                    for cp in range(2):        # sq column pair: cols cp*1024 +: 1024
                        for h in range(HG):
                            if h < 2:
                                kt_h = kt_a[h * 64 : (h + 1) * 64, :]
                                qt_h = qt_a[h * 64 : (h + 1) * 64, :]
                                ctx_h = ctx_a[h * 64 : (h + 1) * 64, :]
                            else:
                                kt_h = kt_b[0:64, :]
                                qt_h = qt_b[0:64, :]
                                ctx_h = ctx_b[0:64, :]

                            cps = [ctx_ps.tile([65, 512], f32, tag="ctx",
                                               name=f"cps_{cp}_{h}_{j}")
                                   for j in range(2)]
                            for i in range(SKC):
                                sp = sc_ps.tile([P, 1024], f32, tag="sc",
                                                name=f"sp_{cp}_{h}_{i}")
                                for j in range(2):
                                    nc.tensor.matmul(
                                        sp[:, j * 512 : (j + 1) * 512],
                                        kt_h[:, i * P : (i + 1) * P],
                                        qt_h[:, cp * 1024 + j * 512 : cp * 1024 + (j + 1) * 512],
                                        start=True, stop=True,
                                    )
                                et = work.tile([P, 1024], f16, tag="exp",
                                               name=f"et_{cp}_{h}_{i}")
                                nc.scalar.activation(et[:], sp[:], Exp, scale=SCALE)
                                if cp == 0 and h == 0:
                                    v_proj(i, ctx_ps)
                                if cp == 0 and h == 1 and i % 2 == 0:
                                    m1_proj(i // 2, ctx_ps)
                                for j in range(2):
                                    nc.tensor.matmul(
                                        cps[j][:],
                                        v_sb[i][:, h, :],
                                        et[:, j * 512 : (j + 1) * 512],
                                        start=(i == 0), stop=(i == SKC - 1),
                                    )
                            for j in range(2):
                                c = cp * 2 + j
                                den = small.tile([1, 512], f32, tag="den")
                                nc.vector.tensor_copy(den[:], cps[j][64:65, :])
                                r = small.tile([1, 512], f32, tag="r")
                                nc.vector.reciprocal_approx_fast(r[:], den[:])
                                rb = small.tile([64, 512], f32, tag="rb")
                                nc.gpsimd.partition_broadcast(rb[:], r[:])
                                nc.vector.tensor_mul(
                                    ctx_h[:, c * 512 : (c + 1) * 512],
                                    cps[j][0:64, :],
                                    rb[:],
                                )
                                if h == HG - 1:
                                    # output projection for this sq chunk
                                    # (all heads done)
                                    for e in range(KD):
                                        op = ctx_ps.tile([P, 512], f32, tag="ctx",
                                                         name=f"op_{e}_{c}")
                                        nc.tensor.matmul(
                                            op[:],
                                            wo_a[:, e * P : (e + 1) * P],
                                            ctx_a[:, c * 512 : (c + 1) * 512],
                                            start=True, stop=False,
                                        )
                                        nc.tensor.matmul(
                                            op[:],
                                            wo_b[:, e * P : (e + 1) * P],
                                            ctx_b[:, c * 512 : (c + 1) * 512],
                                            start=False, stop=True,
                                        )
                                        o = work.tile([P, 512], f32, tag="o",
                                                      bufs=6)
                                        if cp == 1 and e % 2 == 1:
                                            nc.scalar.activation(
                                                o[:], op[:],
                                                mybir.ActivationFunctionType.Copy,
                                            )
                                        else:
                                            nc.vector.tensor_copy(o[:], op[:])
                                        nc.sync.dma_start(
                                            out=outT_d[e][:, c * 512 : (c + 1) * 512],
                                            in_=o[:],
                                        )

    nc.finalize()
    return nc


def _get_nc(body_reps=1):
    key = ("nc", body_reps)
    if key not in _NC_CACHE:
        _NC_CACHE[key] = _build_bass(body_reps)
    return _NC_CACHE[key]


def _core_inputs(c, x, w_q, b_q, w_k, b_k, w_v, b_v, w_o):
    b, g = divmod(c, 4)
    gs = slice(g * E, (g + 1) * E)
    wqk = np.concatenate(
        [np.ascontiguousarray(w_q[gs, :].T), np.ascontiguousarray(w_k[gs, :].T)],
        axis=1,
    )
    bqkv = np.concatenate([b_q[gs], b_k[gs], b_v[gs]]).reshape(1, 3 * E)
    return {
        "xT": np.ascontiguousarray(x[b].T).astype(np.float16),
        "wqkT": wqk.astype(np.float16),
        "wvT": np.ascontiguousarray(w_v[gs, :].T).astype(np.float16),
        "bqkv": bqkv.astype(np.float16),
        "woT": np.ascontiguousarray(w_o[:, gs].T).astype(np.float16),
        "ones": np.ones((P, 512), np.float16),
    }


def kernel(x, w_q, b_q, w_k, b_k, w_v, b_v, w_o, b_o, _trace=False, _debug=False):
    from concourse.bass_utils import run_bass_kernel_spmd

    x = np.asarray(x, np.float32)
    args = [np.asarray(a, np.float32) for a in
            (w_q, b_q, w_k, b_k, w_v, b_v, w_o)]
    b_o = np.asarray(b_o, np.float32)

    nc = _get_nc()
    in_maps = [_core_inputs(c, x, *args) for c in range(8)]
    res = run_bass_kernel_spmd(nc, in_maps, core_ids=list(range(8)), trace=_trace)

    out = np.zeros((B, S, D), np.float32)
    for c in range(8):
        out[c // 4] += res.results[c]["outT"].T
    out += b_o
    if _trace:
        kernel._last_results = res
    return out


# revision 6
# speedup vs baseline: 1.4516x; 1.0372x over previous
"""Multi-head attention (B=2, S=2048, D=768, H=12) on 8 NeuronCores.

Sharding: data-parallel over batch (2) x tensor-parallel over heads (4 groups
of 3 heads) = 8 cores. Each core computes its 3 heads' Q/K/V projections,
attention, and a partial output projection; the host sums the 4 per-batch
partials and adds the output bias.

All SBUF operands are fp16 (PE fast mode + FWL; PSUM accumulation stays
fp32). The schedule keeps ScalarE (the Exp pipeline, ~110us floor) busy
end-to-end and hides everything else in the PE's slack behind it:
  - two parallel DMA rings: weights on the sync-engine ring, x on the
    scalar-engine ring (both HWDGE), each as one batched descriptor set
  - K m=0 tiles project before Q so head-0 scores can issue the moment the
    first qt column pair lands; the m=1 (head 2) Q/K tiles and the V
    projection are interleaved into the head-0/1 attention loops, drawing
    PSUM from the ctx pool's spare slots
  - attention is split by sq column pairs (2x 1024 cols): scoresT
    [sk 128, 1024] per (cpair, head, sk-chunk) in PSUM -> one Exp on
    ScalarE (scale folded in) -> ctx accumulation [65, 512] with a ones
    column in V giving softmax denominators for free
  - the cpair-0 output projection is interleaved into cpair-1 head-0
    iterations so the PE never stalls the exp pipeline; only the cpair-1
    projection remains as a short tail
  outT [768, 2048] fp16 partial output projection, host-summed across
    head groups in fp32
"""

import sys

sys.path.insert(0, "/opt/trn_rl_repo")

import numpy as np

B, S, D = 2, 2048, 768
H, DK = 12, 64
P = 128
HG = 3              # heads per core
E = HG * DK         # 192: per-core projection width
KD = D // P         # 6 contraction chunks
SQC = S // 512      # 4 sq chunks of 512
SKC = S // P        # 16 sk chunks of 128
SCALE = 1.0 / 8.0   # 1/sqrt(DK)

_NC_CACHE = {}


def _build_bass(body_reps=1):
    import concourse.bacc as bacc
    import concourse.tile as tile
    from concourse import mybir

    f16 = mybir.dt.float16
    f32 = mybir.dt.float32
    Exp = mybir.ActivationFunctionType.Exp

    nc = bacc.Bacc(trn_type="TRN2", debug=False)

    xT = nc.dram_tensor("xT", [D, S], f16, kind="ExternalInput")
    wqkT = nc.dram_tensor("wqkT", [D, 2 * E], f16, kind="ExternalInput")
    wvT = nc.dram_tensor("wvT", [D, E], f16, kind="ExternalInput")
    bqkv = nc.dram_tensor("bqkv", [1, 3 * E], f16, kind="ExternalInput")
    woT = nc.dram_tensor("woT", [E, D], f16, kind="ExternalInput")
    ones_d = nc.dram_tensor("ones", [P, 512], f16, kind="ExternalInput")
    outT = nc.dram_tensor("outT", [D, S], f16, kind="ExternalOutput")

    xT_d = xT.ap().rearrange("(c p) s -> p c s", p=P)
    wqkT_d = wqkT.ap().rearrange("(c p) e -> p c e", p=P)
    wvT_d = wvT.ap().rearrange("(c p) e -> p c e", p=P)
    outT_d = outT.ap().rearrange("(c p) s -> p c s", p=P)

    with tile.TileContext(nc) as tc:
        for _rep in range(body_reps):
            with tc.tile_pool(name="persist", bufs=1) as persist, \
                 tc.tile_pool(name="work", bufs=4) as work, \
                 tc.tile_pool(name="small", bufs=2) as small:

                # ---- batched input DMAs on two parallel HWDGE rings ----
                wqk = persist.tile([P, KD, 2 * E], f16, tag="wqk")
                nc.sync.dma_start(out=wqk[:], in_=wqkT_d)
                bqkv_sb = persist.tile([1, 3 * E], f16, tag="bqkv")
                nc.sync.dma_start(out=bqkv_sb[:], in_=bqkv.ap())
                ones = persist.tile([P, 512], f16, tag="ones")
                nc.sync.dma_start(out=ones[:], in_=ones_d.ap())
                wv = persist.tile([P, KD, E], f16, tag="wv")
                nc.sync.dma_start(out=wv[:], in_=wvT_d)
                wo_a = persist.tile([P, D], f16, tag="wo_a")
                nc.sync.dma_start(out=wo_a[:], in_=woT.ap()[0:P, :])
                wo_b = persist.tile([64, D], f16, tag="wo_b")
                nc.sync.dma_start(out=wo_b[:], in_=woT.ap()[P:E, :])

                x_all = persist.tile([P, KD, S], f16, tag="x")
                nc.scalar.dma_start(out=x_all[:, 0:3, :], in_=xT_d[:, 0:3, :])
                nc.scalar.dma_start(out=x_all[:, 3:KD, :], in_=xT_d[:, 3:KD, :])

                # preload the Exp table while the x DMAs run
                warm = small.tile([1, 16], f16, tag="warm")
                nc.scalar.activation(warm[:], ones[0:1, 0:16], Exp, scale=1.0)

                # ---- persistent activations ----
                # qt split per column pair so cp0 attention doesn't wait on
                # the cp1 eviction
                qt_a = [persist.tile([P, 1024], f16, tag=f"qt_a{cp}",
                                     name=f"qt_a{cp}") for cp in range(2)]
                qt_b = [persist.tile([64, 1024], f16, tag=f"qt_b{cp}",
                                     name=f"qt_b{cp}") for cp in range(2)]
                kt_a = persist.tile([P, S], f16, tag="kt_a")
                kt_b = persist.tile([64, S], f16, tag="kt_b")
                v_sb = [persist.tile([P, HG, 65], f16, tag=f"v{i}", name=f"v{i}")
                        for i in range(SKC)]
                ctx_a = persist.tile([P, S], f16, tag="ctx_a")
                ctx_b = persist.tile([64, S], f16, tag="ctx_b")

                def w_slice(d, which, m, mw):
                    off = 0 if which == 0 else E
                    return wqk[:, d, off + m * P : off + m * P + mw]

                def b_slice(which, m, mw):
                    off = which * E
                    return bqkv_sb[0:1, off + m * P : off + m * P + mw]

                # ====== Q/K m=0 projections (heads 0,1), K first ======
                # d=0..2 passes first (first x DMA piece), then d=3..5.
                with tc.tile_pool(name="proj_ps", bufs=8, space="PSUM") as proj_ps:
                    ps = []
                    for which in (1, 0):     # K tiles first, then Q
                        for c in range(SQC):
                            ps.append(proj_ps.tile(
                                [P, 512], f32, tag="proj",
                                name=f"proj_{which}_{c}"))
                    for dlo, dhi in ((0, 3), (3, KD)):
                        k = 0
                        for which in (1, 0):
                            for c in range(SQC):
                                for d in range(dlo, dhi):
                                    nc.tensor.matmul(
                                        ps[k][:],
                                        w_slice(d, which, 0, P),
                                        x_all[:, d, c * 512 : (c + 1) * 512],
                                        start=(d == 0), stop=False,
                                    )
                                k += 1
                    k = 0
                    for which in (1, 0):
                        for c in range(SQC):
                            nc.tensor.matmul(
                                ps[k][:],
                                b_slice(which, 0, P),
                                ones[0:1, 0:512],
                                start=False, stop=True,
                            )
                            if which == 1:
                                dst = kt_a[:, c * 512 : (c + 1) * 512]
                            else:
                                dst = qt_a[c // 2][:, (c % 2) * 512 : (c % 2) * 512 + 512]
                            nc.vector.tensor_copy(dst, ps[k][:])
                            k += 1

                # ====== attention ======
                # PE slack behind the ACT-bound exp pipeline absorbs the V
                # projection (cp0 head-0 loop), the m=1 Q/K projection (cp0
                # head-1 loop), and the cp0 output projection (cp1 head-0
                # loop).
                def v_proj(i, pool):
                    vps = pool.tile([P, E], f32, tag="ctx", name=f"vps_{i}")
                    for d in range(KD):
                        nc.tensor.matmul(
                            vps[:],
                            x_all[:, d, i * P : (i + 1) * P],
                            wv[:, d, :],
                            start=(d == 0), stop=False,
                        )
                    nc.tensor.matmul(
                        vps[:], ones[0:1, 0:P], b_slice(2, 0, E),
                        start=False, stop=True,
                    )
                    nc.vector.tensor_copy(
                        v_sb[i][:, :, 64:65], ones[:, 0:3][:, :, None]
                    )
                    nc.vector.tensor_copy(
                        v_sb[i][:, :, 0:64],
                        vps[:, 0:E].rearrange("p (h d) -> p h d", h=HG),
                    )

                def m1_proj(t, pool):
                    # t in 0..7: K m=1 tiles (c=0..3) then Q m=1 tiles
                    which, c = divmod(t, SQC)
                    which = 1 - which        # K first
                    mp = pool.tile([64, 512], f32, tag="ctx", name=f"m1_{t}")
                    for d in range(KD):
                        nc.tensor.matmul(
                            mp[:],
                            w_slice(d, which, 1, 64),
                            x_all[:, d, c * 512 : (c + 1) * 512],
                            start=(d == 0), stop=False,
                        )
                    nc.tensor.matmul(
                        mp[:], b_slice(which, 1, 64), ones[0:1, 0:512],
                        start=False, stop=True,
                    )
                    if which == 1:
                        dst = kt_b[:, c * 512 : (c + 1) * 512]
                    else:
                        dst = qt_b[c // 2][:, (c % 2) * 512 : (c % 2) * 512 + 512]
                    nc.vector.tensor_copy(dst, mp[:])

                def out_proj_unit(c, ep, pool, use_act):
                    # output projection for sq chunk c, e-pair ep
                    o = work.tile([P, 2, 512], f16, tag="o", bufs=4,
                                  name=f"o_{c}_{ep}")
                    for k in range(2):
                        e = 2 * ep + k
                        op = pool.tile([P, 512], f32, tag="ctx",
                                       name=f"op_{e}_{c}")
                        nc.tensor.matmul(
                            op[:],
                            wo_a[:, e * P : (e + 1) * P],
                            ctx_a[:, c * 512 : (c + 1) * 512],
                            start=True, stop=False,
                        )
                        nc.tensor.matmul(
                            op[:],
                            wo_b[:, e * P : (e + 1) * P],
                            ctx_b[:, c * 512 : (c + 1) * 512],
                            start=False, stop=True,
                        )
                        if use_act and k % 2 == 1:
                            nc.scalar.activation(
                                o[:, k, :], op[:],
                                mybir.ActivationFunctionType.Copy,
                            )
                        else:
                            nc.vector.tensor_copy(o[:, k, :], op[:])
                    nc.sync.dma_start(
                        out=outT_d[:, 2 * ep : 2 * ep + 2,
                                   c * 512 : (c + 1) * 512],
                        in_=o[:],
                    )

                with tc.tile_pool(name="sc_ps", bufs=2, space="PSUM") as sc_ps, \
                     tc.tile_pool(name="ctx_ps", bufs=4, space="PSUM") as ctx_ps:
                    for cp in range(2):        # sq column pair: cols cp*1024 +: 1024
                        for h in range(HG):
                            if h < 2:
                                kt_h = kt_a[h * 64 : (h + 1) * 64, :]
                                qt_h = qt_a[cp][h * 64 : (h + 1) * 64, :]
                                ctx_h = ctx_a[h * 64 : (h + 1) * 64, :]
                            else:
                                kt_h = kt_b[0:64, :]
                                qt_h = qt_b[cp][0:64, :]
                                ctx_h = ctx_b[0:64, :]

                            cps = [ctx_ps.tile([65, 512], f32, tag="ctx",
                                               name=f"cps_{cp}_{h}_{j}")
                                   for j in range(2)]
                            for i in range(SKC):
                                sp = sc_ps.tile([P, 1024], f32, tag="sc",
                                                name=f"sp_{cp}_{h}_{i}")
                                for j in range(2):
                                    nc.tensor.matmul(
                                        sp[:, j * 512 : (j + 1) * 512],
                                        kt_h[:, i * P : (i + 1) * P],
                                        qt_h[:, j * 512 : (j + 1) * 512],
                                        start=True, stop=True,
                                    )
                                et = work.tile([P, 1024], f16, tag="exp",
                                               name=f"et_{cp}_{h}_{i}")
                                nc.scalar.activation(et[:], sp[:], Exp, scale=SCALE)
                                if cp == 0 and h == 0:
                                    v_proj(i, ctx_ps)
                                if cp == 0 and h == 1 and i % 2 == 0:
                                    m1_proj(i // 2, ctx_ps)
                                if cp == 1 and h == 0 and i % 2 == 0 and i < 12:
                                    # cp0 output projection: 6 units over 12 iters
                                    t = i // 2
                                    out_proj_unit(t // 3, t % 3, ctx_ps, False)
                                for j in range(2):
                                    nc.tensor.matmul(
                                        cps[j][:],
                                        v_sb[i][:, h, :],
                                        et[:, j * 512 : (j + 1) * 512],
                                        start=(i == 0), stop=(i == SKC - 1),
                                    )
                            for j in range(2):
                                c = cp * 2 + j
                                den = small.tile([1, 512], f32, tag="den")
                                nc.vector.tensor_copy(den[:], cps[j][64:65, :])
                                r = small.tile([1, 512], f32, tag="r")
                                nc.vector.reciprocal_approx_fast(r[:], den[:])
                                rb = small.tile([64, 512], f32, tag="rb")
                                nc.gpsimd.partition_broadcast(rb[:], r[:])
                                nc.vector.tensor_mul(
                                    ctx_h[:, c * 512 : (c + 1) * 512],
                                    cps[j][0:64, :],
                                    rb[:],
                                )
                                if cp == 1 and h == HG - 1:
                                    # cp1 output projection tail
                                    for ep in range(3):
                                        out_proj_unit(c, ep, ctx_ps, True)

    nc.finalize()
    return nc


def _get_nc(body_reps=1):
    key = ("nc", body_reps)
    if key not in _NC_CACHE:
        _NC_CACHE[key] = _build_bass(body_reps)
    return _NC_CACHE[key]


def _core_inputs(c, x, w_q, b_q, w_k, b_k, w_v, b_v, w_o):
    b, g = divmod(c, 4)
    gs = slice(g * E, (g + 1) * E)
    wqk = np.concatenate(
        [np.ascontiguousarray(w_q[gs, :].T), np.ascontiguousarray(w_k[gs, :].T)],
        axis=1,
    )
    bqkv = np.concatenate([b_q[gs], b_k[gs], b_v[gs]]).reshape(1, 3 * E)
    return {
        "xT": np.ascontiguousarray(x[b].T).astype(np.float16),
        "wqkT": wqk.astype(np.float16),
        "wvT": np.ascontiguousarray(w_v[gs, :].T).astype(np.float16),
        "bqkv": bqkv.astype(np.float16),
        "woT": np.ascontiguousarray(w_o[:, gs].T).astype(np.float16),
        "ones": np.ones((P, 512), np.float16),
    }


def kernel(x, w_q, b_q, w_k, b_k, w_v, b_v, w_o, b_o, _trace=False, _debug=False):
    from concourse.bass_utils import run_bass_kernel_spmd

    x = np.asarray(x, np.float32)
    args = [np.asarray(a, np.float32) for a in
            (w_q, b_q, w_k, b_k, w_v, b_v, w_o)]
    b_o = np.asarray(b_o, np.float32)

    nc = _get_nc()
    in_maps = [_core_inputs(c, x, *args) for c in range(8)]
    res = run_bass_kernel_spmd(nc, in_maps, core_ids=list(range(8)), trace=_trace)

    out = np.zeros((B, S, D), np.float32)
    for c in range(8):
        out[c // 4] += res.results[c]["outT"].astype(np.float32).T
    out += b_o
    if _trace:
        kernel._last_results = res
    return out


# revision 8
# speedup vs baseline: 1.6654x; 1.1472x over previous
"""Multi-head attention (B=2, S=2048, D=768, H=12) on 8 NeuronCores.

Sharding: data-parallel over batch (2) x tensor-parallel over heads (4 groups
of 3 heads) = 8 cores. Each core computes its 3 heads' Q/K/V projections,
attention, and a partial output projection; the host sums the 4 per-batch
partials and adds the output bias.

All SBUF operands are fp16 (PE fast mode + FWL; PSUM accumulation stays
fp32). The schedule keeps ScalarE (the Exp pipeline, ~110us floor) busy
end-to-end and hides everything else in the PE's slack behind it:
  - inputs are host-permuted to [partition, chunk, ...] layouts so every
    DMA is a contiguous large-descriptor transfer; weights ride the
    sync-engine HWDGE ring while x rides the scalar-engine ring in two
    pieces, overlapping the first Q/K matmuls
  - K m=0 tiles project before Q, each tile evicting immediately after its
    last accumulation matmul, so head-0 scores chain on with no PE gap
    (keeping the HAM clock warm into attention)
  - the m=1 (head 2) Q/K tiles run as column-tiled pairs (Q in array cols
    0-63, K in 64-127, concurrently) inside the head-1 loop; the V
    projection rides the head-0 loop; the cpair-0 output projection rides
    the cpair-1 head-0 loop
  - attention is split by sq column pairs (2x 1024 cols): scoresT
    [sk 128, 1024] per (cpair, head, sk-chunk) in PSUM -> one Exp on
    ScalarE (scale folded in) -> ctx accumulation [65, 512] with a ones
    column in V giving softmax denominators for free; normalization uses a
    PE rank-1 broadcast of the reciprocal row (no GpSimd in the chain)
  outT [768, 2048] fp16 partial output projection, host-summed across
    head groups in fp32
"""

import sys

sys.path.insert(0, "/opt/trn_rl_repo")

import numpy as np

B, S, D = 2, 2048, 768
H, DK = 12, 64
P = 128
HG = 3              # heads per core
E = HG * DK         # 192: per-core projection width
KD = D // P         # 6 contraction chunks
SQC = S // 512      # 4 sq chunks of 512
SKC = S // P        # 16 sk chunks of 128
SCALE = 1.0 / 8.0   # 1/sqrt(DK)

_NC_CACHE = {}


def _build_bass(body_reps=1):
    import concourse.bacc as bacc
    import concourse.tile as tile
    from concourse import mybir

    f16 = mybir.dt.float16
    f32 = mybir.dt.float32
    Exp = mybir.ActivationFunctionType.Exp

    nc = bacc.Bacc(trn_type="TRN2", debug=False)

    # host-permuted: row p holds [KD, ...] chunk-contiguous data
    xP = nc.dram_tensor("xP", [P, KD * S], f16, kind="ExternalInput")
    wP = nc.dram_tensor("wP", [P, KD * 3 * E], f16, kind="ExternalInput")
    bqkv = nc.dram_tensor("bqkv", [1, 3 * E], f16, kind="ExternalInput")
    woT = nc.dram_tensor("woT", [E, D], f16, kind="ExternalInput")
    ones_d = nc.dram_tensor("ones", [P, 512], f16, kind="ExternalInput")
    outT = nc.dram_tensor("outT", [D, S], f16, kind="ExternalOutput")

    xP_d = xP.ap().rearrange("p (c s) -> p c s", s=S)
    wP_d = wP.ap().rearrange("p (c e) -> p c e", e=3 * E)
    outT_d = outT.ap().rearrange("(c p) s -> p c s", p=P)

    with tile.TileContext(nc) as tc:
        for _rep in range(body_reps):
            with tc.tile_pool(name="persist", bufs=1) as persist, \
                 tc.tile_pool(name="work", bufs=4) as work, \
                 tc.tile_pool(name="small", bufs=2) as small:

                # ---- batched input DMAs on two parallel HWDGE rings ----
                wqkv = persist.tile([P, KD, 3 * E], f16, tag="wqkv")
                nc.sync.dma_start(out=wqkv[:], in_=wP_d)
                bqkv_sb = persist.tile([1, 3 * E], f16, tag="bqkv")
                nc.sync.dma_start(out=bqkv_sb[:], in_=bqkv.ap())
                ones = persist.tile([P, 512], f16, tag="ones")
                nc.sync.dma_start(out=ones[:], in_=ones_d.ap())
                wo_a = persist.tile([P, D], f16, tag="wo_a")
                nc.sync.dma_start(out=wo_a[:], in_=woT.ap()[0:P, :])
                wo_b = persist.tile([64, D], f16, tag="wo_b")
                nc.sync.dma_start(out=wo_b[:], in_=woT.ap()[P:E, :])

                x_all = persist.tile([P, KD, S], f16, tag="x")
                nc.scalar.dma_start(out=x_all[:, 0:3, :], in_=xP_d[:, 0:3, :])
                nc.scalar.dma_start(out=x_all[:, 3:KD, :], in_=xP_d[:, 3:KD, :])

                # preload the Exp table while the x DMAs run
                warm = small.tile([1, 16], f16, tag="warm")
                nc.scalar.activation(warm[:], ones[0:1, 0:16], Exp, scale=1.0)

                # ---- persistent activations ----
                # qt split per column pair so cp0 attention doesn't wait on
                # the cp1 eviction
                qt_a = [persist.tile([P, 1024], f16, tag=f"qt_a{cp}",
                                     name=f"qt_a{cp}") for cp in range(2)]
                qt_b = [persist.tile([64, 1024], f16, tag=f"qt_b{cp}",
                                     name=f"qt_b{cp}") for cp in range(2)]
                kt_a = persist.tile([P, S], f16, tag="kt_a")
                kt_b = persist.tile([64, S], f16, tag="kt_b")
                v_sb = [persist.tile([P, HG, 65], f16, tag=f"v{i}", name=f"v{i}")
                        for i in range(SKC)]
                ctx_a = persist.tile([P, S], f16, tag="ctx_a")
                ctx_b = persist.tile([64, S], f16, tag="ctx_b")

                def w_slice(d, which, m, mw):
                    off = which * E
                    return wqkv[:, d, off + m * P : off + m * P + mw]

                def b_slice(which, m, mw):
                    off = which * E
                    return bqkv_sb[0:1, off + m * P : off + m * P + mw]

                def qdst(c):
                    return qt_a[c // 2][:, (c % 2) * 512 : (c % 2) * 512 + 512]

                # ====== Q/K m=0 projections (heads 0,1), K first ======
                # pass 1: d=0..2 accumulation for all 8 tiles (first x piece)
                # pass 2: per tile d=3..5 + bias + immediate eviction, K tiles
                # first, so attention chains on with no PE gap.
                with tc.tile_pool(name="proj_ps", bufs=8, space="PSUM") as proj_ps:
                    ps = []
                    for which in (1, 0):     # K tiles first, then Q
                        for c in range(SQC):
                            ps.append(proj_ps.tile(
                                [P, 512], f32, tag="proj",
                                name=f"proj_{which}_{c}"))
                    k = 0
                    for which in (1, 0):
                        for c in range(SQC):
                            for d in range(3):
                                nc.tensor.matmul(
                                    ps[k][:],
                                    w_slice(d, which, 0, P),
                                    x_all[:, d, c * 512 : (c + 1) * 512],
                                    start=(d == 0), stop=False,
                                )
                            k += 1
                    k = 0
                    for which in (1, 0):
                        for c in range(SQC):
                            for d in range(3, KD):
                                nc.tensor.matmul(
                                    ps[k][:],
                                    w_slice(d, which, 0, P),
                                    x_all[:, d, c * 512 : (c + 1) * 512],
                                    start=False, stop=False,
                                )
                            nc.tensor.matmul(
                                ps[k][:],
                                b_slice(which, 0, P),
                                ones[0:1, 0:512],
                                start=False, stop=True,
                            )
                            dst = kt_a[:, c * 512 : (c + 1) * 512] \
                                if which == 1 else qdst(c)
                            nc.vector.tensor_copy(dst, ps[k][:])
                            k += 1

                # ====== attention ======
                def v_proj(i, pool):
                    vps = pool.tile([P, E], f32, tag="ctx", name=f"vps_{i}")
                    for d in range(KD):
                        nc.tensor.matmul(
                            vps[:],
                            x_all[:, d, i * P : (i + 1) * P],
                            wv_col(d),
                            start=(d == 0), stop=False,
                        )
                    nc.tensor.matmul(
                        vps[:], ones[0:1, 0:P], b_slice(2, 0, E),
                        start=False, stop=True,
                    )
                    nc.vector.tensor_copy(
                        v_sb[i][:, :, 64:65], ones[:, 0:3][:, :, None]
                    )
                    nc.vector.tensor_copy(
                        v_sb[i][:, :, 0:64],
                        vps[:, 0:E].rearrange("p (h d) -> p h d", h=HG),
                    )

                def wv_col(d):
                    return wqkv[:, d, 2 * E : 3 * E]

                def m1_pair(c, pool):
                    # Q and K m=1 tiles for sq chunk c, column-tiled into one
                    # concurrent array pass: Q -> cols 0-63, K -> cols 64-127
                    mp = pool.tile([P, 512], f32, tag="ctx", name=f"m1_{c}")
                    for d in range(KD):
                        rhs = x_all[:, d, c * 512 : (c + 1) * 512]
                        nc.tensor.matmul(
                            mp[0:64, :], w_slice(d, 0, 1, 64), rhs,
                            start=(d == 0), stop=False, tile_position=(0, 0),
                        )
                        nc.tensor.matmul(
                            mp[64:128, :], w_slice(d, 1, 1, 64), rhs,
                            start=(d == 0), stop=False, tile_position=(0, 64),
                        )
                    nc.tensor.matmul(
                        mp[0:64, :], b_slice(0, 1, 64), ones[0:1, 0:512],
                        start=False, stop=True, tile_position=(0, 0),
                    )
                    nc.tensor.matmul(
                        mp[64:128, :], b_slice(1, 1, 64), ones[0:1, 0:512],
                        start=False, stop=True, tile_position=(0, 64),
                    )
                    nc.vector.tensor_copy(
                        qt_b[c // 2][:, (c % 2) * 512 : (c % 2) * 512 + 512],
                        mp[0:64, :])
                    nc.vector.tensor_copy(
                        kt_b[:, c * 512 : (c + 1) * 512], mp[64:128, :])

                def out_proj_unit(c, ep, pool, use_act):
                    # output projection for sq chunk c, e-pair ep
                    o = work.tile([P, 2, 512], f16, tag="o", bufs=4,
                                  name=f"o_{c}_{ep}")
                    for k in range(2):
                        e = 2 * ep + k
                        op = pool.tile([P, 512], f32, tag="ctx",
                                       name=f"op_{e}_{c}")
                        nc.tensor.matmul(
                            op[:],
                            wo_a[:, e * P : (e + 1) * P],
                            ctx_a[:, c * 512 : (c + 1) * 512],
                            start=True, stop=False,
                        )
                        nc.tensor.matmul(
                            op[:],
                            wo_b[:, e * P : (e + 1) * P],
                            ctx_b[:, c * 512 : (c + 1) * 512],
                            start=False, stop=True,
                        )
                        if use_act and k % 2 == 1:
                            nc.scalar.activation(
                                o[:, k, :], op[:],
                                mybir.ActivationFunctionType.Copy,
                            )
                        else:
                            nc.vector.tensor_copy(o[:, k, :], op[:])
                    nc.sync.dma_start(
                        out=outT_d[:, 2 * ep : 2 * ep + 2,
                                   c * 512 : (c + 1) * 512],
                        in_=o[:],
                    )

                with tc.tile_pool(name="sc_ps", bufs=2, space="PSUM") as sc_ps, \
                     tc.tile_pool(name="ctx_ps", bufs=4, space="PSUM") as ctx_ps:
                    for cp in range(2):        # sq column pair: cols cp*1024 +: 1024
                        for h in range(HG):
                            if h < 2:
                                kt_h = kt_a[h * 64 : (h + 1) * 64, :]
                                qt_h = qt_a[cp][h * 64 : (h + 1) * 64, :]
                                ctx_h = ctx_a[h * 64 : (h + 1) * 64, :]
                            else:
                                kt_h = kt_b[0:64, :]
                                qt_h = qt_b[cp][0:64, :]
                                ctx_h = ctx_b[0:64, :]

                            cps = [ctx_ps.tile([65, 512], f32, tag="ctx",
                                               name=f"cps_{cp}_{h}_{j}")
                                   for j in range(2)]
                            for i in range(SKC):
                                sp = sc_ps.tile([P, 1024], f32, tag="sc",
                                                name=f"sp_{cp}_{h}_{i}")
                                for j in range(2):
                                    nc.tensor.matmul(
                                        sp[:, j * 512 : (j + 1) * 512],
                                        kt_h[:, i * P : (i + 1) * P],
                                        qt_h[:, j * 512 : (j + 1) * 512],
                                        start=True, stop=True,
                                    )
                                et = work.tile([P, 1024], f16, tag="exp",
                                               name=f"et_{cp}_{h}_{i}")
                                nc.scalar.activation(et[:], sp[:], Exp, scale=SCALE)
                                if cp == 0 and h == 0:
                                    v_proj(i, ctx_ps)
                                if cp == 0 and h == 1 and i % 4 == 0:
                                    m1_pair(i // 4, ctx_ps)
                                if cp == 1 and h == 0 and i % 2 == 0 \
                                        and 2 <= i < 14:
                                    # cp0 output projection: 6 units
                                    t = i // 2 - 1
                                    out_proj_unit(t // 3, t % 3, ctx_ps, False)
                                for j in range(2):
                                    nc.tensor.matmul(
                                        cps[j][:],
                                        v_sb[i][:, h, :],
                                        et[:, j * 512 : (j + 1) * 512],
                                        start=(i == 0), stop=(i == SKC - 1),
                                    )
                            for j in range(2):
                                c = cp * 2 + j
                                den = small.tile([1, 512], f32, tag="den")
                                nc.vector.tensor_copy(den[:], cps[j][64:65, :])
                                r = small.tile([1, 512], f32, tag="r")
                                nc.vector.reciprocal_approx_fast(r[:], den[:])
                                rb = small.tile([64, 512], f32, tag="rb")
                                nc.gpsimd.partition_broadcast(rb[:], r[:])
                                nc.vector.tensor_mul(
                                    ctx_h[:, c * 512 : (c + 1) * 512],
                                    cps[j][0:64, :],
                                    rb[:],
                                )
                                if cp == 1 and h == HG - 1:
                                    # cp1 output projection tail
                                    for ep in range(3):
                                        out_proj_unit(c, ep, ctx_ps, True)

    nc.finalize()
    return nc


def _get_nc(body_reps=1):
    key = ("nc", body_reps)
    if key not in _NC_CACHE:
        _NC_CACHE[key] = _build_bass(body_reps)
    return _NC_CACHE[key]


def _core_inputs(c, x, w_q, b_q, w_k, b_k, w_v, b_v, w_o):
    b, g = divmod(c, 4)
    gs = slice(g * E, (g + 1) * E)
    # [768, 3E] (q|k|v columns) -> [P, KD*3E]: row p holds chunk-major data
    wqkv = np.concatenate(
        [w_q[gs, :].T, w_k[gs, :].T, w_v[gs, :].T], axis=1
    ).astype(np.float16)
    wP = np.ascontiguousarray(
        wqkv.reshape(KD, P, 3 * E).transpose(1, 0, 2)).reshape(P, KD * 3 * E)
    xT = x[b].T.astype(np.float16)   # [768, 2048]
    xP = np.ascontiguousarray(
        xT.reshape(KD, P, S).transpose(1, 0, 2)).reshape(P, KD * S)
    bqkv = np.concatenate([b_q[gs], b_k[gs], b_v[gs]]).reshape(1, 3 * E)
    return {
        "xP": xP,
        "wP": wP,
        "bqkv": bqkv.astype(np.float16),
        "woT": np.ascontiguousarray(w_o[:, gs].T).astype(np.float16),
        "ones": np.ones((P, 512), np.float16),
    }


def kernel(x, w_q, b_q, w_k, b_k, w_v, b_v, w_o, b_o, _trace=False, _debug=False):
    from concourse.bass_utils import run_bass_kernel_spmd

    x = np.asarray(x, np.float32)
    args = [np.asarray(a, np.float32) for a in
            (w_q, b_q, w_k, b_k, w_v, b_v, w_o)]
    b_o = np.asarray(b_o, np.float32)

    nc = _get_nc()
    in_maps = [_core_inputs(c, x, *args) for c in range(8)]
    res = run_bass_kernel_spmd(nc, in_maps, core_ids=list(range(8)), trace=_trace)

    out = np.zeros((B, S, D), np.float32)
    for c in range(8):
        out[c // 4] += res.results[c]["outT"].astype(np.float32).T
    out += b_o
    if _trace:
        kernel._last_results = res
    return out
